# revision 13
# baseline (speedup 1.0000x reference)
"""GAT (2-layer, 4-head then 1-head) + global mean pool + FC on 8 NeuronCores.

Strategy (1D node partition, per sharding hint):
  - Nodes are split into 8 contiguous shards of 1250; each core owns the
    edges whose *destination* falls in its shard (segment softmax and
    aggregation are per-destination, so each core's work is independent
    given the full feature table).
  - Launch 1: every core computes the full H1'' = x @ [W1 | W1@A1]
    (features + per-node attention logits alpha_src/alpha_dst), then runs
    the edge pass for its dst shard: per 128-edge chunk it row-gathers
    H1''[src] via indirect DMA, builds a 0/1 selection matrix S0[e,d] from
    the (host-precomputed) local dst index of each edge, and uses matmuls
    with S0 / S0^T for segment-sum (softmax denominators) and
    segment-broadcast (alpha_dst to edges).  exp() weights are folded into
    the gathered rows and aggregated into PSUM with S0 as stationary;
    the softmax normalization (1/denom) is applied per-partition during
    PSUM eviction, fused with ReLU.
  - Host gathers the 8 relu(out1) shards (pure reshuffling, no math) and
    launch 2 repeats the same for layer 2 (single head), then does
    global-mean-pool + FC as matmuls (graph-id selection matrix built the
    same way); each core emits a partial [64,64] that the host sums.
  - Per-core inputs are node-rolled so each core's shard occupies rows
    0..1249 of its own H'' table (keeps the SPMD program identical).

All device compute (matmuls, softmax, relu, pooling, FC) runs on the
NeuronCores; the host only sorts/pads edge indices, folds constant
parameter products (W@A), and concatenates/sums per-core outputs.
"""

import sys

sys.path.insert(0, "/opt/trn_rl_repo")

import numpy as np
import ml_dtypes

BF = ml_dtypes.bfloat16

from concourse import bacc, bass, mybir, tile
from concourse.bass_utils import run_bass_kernel_spmd
from concourse.masks import make_identity

P = 128
N = 10000
IN = 256
HID = 128
HEADS = 4
OUT = 64
G = 64
NEG = 0.2
NCORES = 8
NSH = N // NCORES  # 1250 nodes per core
TSZ = 125          # dst nodes per tile
NT = NSH // TSZ    # 10 dst tiles per core
F32 = mybir.dt.float32
BF16 = mybir.dt.bfloat16
I32 = mybir.dt.int32
I16 = mybir.dt.int16


def _pad_cols(ncols):
    # bf16 row bytes must be a multiple of 256 for dma_gather
    return ((ncols * 2 + 255) // 256) * 256 // 2


def build_program(C, fcols, nheads, xk, do_pool):
    """Build one SPMD launch. C: edge chunks per dst tile; fcols: feature cols
    (512 or 128); nheads: attention heads; xk: stage-A contraction dim;
    do_pool: append mean-pool + FC partial."""
    ncols = fcols + 2 * nheads  # H'' row: [feats | a_src | a_dst]
    ncolsP = _pad_cols(ncols)   # padded bf16 row
    KCH = xk // P
    nc = bacc.Bacc("TRN2", target_bir_lowering=False, debug=False,
                   num_devices=NCORES)

    xT = nc.dram_tensor("xT", [xk, N], BF16, kind="ExternalInput").ap()
    Wa = nc.dram_tensor("Wa", [xk, ncols], BF16, kind="ExternalInput").ap()
    s0tT = nc.dram_tensor("s0tT", [NT, P, C * P], BF16, kind="ExternalInput").ap()
    srcidx = nc.dram_tensor("srcidx", [NT, P, C], I32, kind="ExternalInput").ap()
    dstcol = nc.dram_tensor("dstcol", [NT, P, C], F32, kind="ExternalInput").ap()
    outsh = nc.dram_tensor("outsh", [NSH, fcols], F32, kind="ExternalOutput").ap()
    hpp = nc.dram_tensor("hpp", [N, ncolsP], BF16).ap()  # internal H'' table
    if do_pool:
        batchcol = nc.dram_tensor("batchcol", [P * NT, 1], F32,
                                  kind="ExternalInput").ap()
        rcounts = nc.dram_tensor("rcounts", [G, 1], F32, kind="ExternalInput").ap()
        fcw = nc.dram_tensor("fcw", [HID, OUT], F32, kind="ExternalInput").ap()
        yout = nc.dram_tensor("yout", [G, OUT], F32, kind="ExternalOutput").ap()

    with tile.TileContext(nc) as tc:
        with tc.tile_pool(name="const", bufs=1) as constp, \
             tc.tile_pool(name="stash", bufs=2) as stashp, \
             tc.tile_pool(name="sb", bufs=2) as sb, \
             tc.tile_pool(name="ps", bufs=2, space="PSUM") as ps, \
             tc.tile_pool(name="psacc", bufs=1, space="PSUM") as psacc:

            ident = constp.tile([P, P], BF16)
            make_identity(nc, ident[:])
            iota_i = constp.tile([P, P], I32)
            nc.gpsimd.iota(iota_i[:], pattern=[[1, P]], base=0, channel_multiplier=0)
            iota_f = constp.tile([P, P], F32)
            nc.vector.tensor_copy(iota_f[:], iota_i[:])

            # ---- stage A: hpp[n, :] = x[n, :] @ Wa  (replicated, all N rows)
            wa_sb = constp.tile([P, KCH, ncols], BF16)
            for k in range(KCH):
                nc.sync.dma_start(out=wa_sb[:, k, :], in_=Wa[k * P:(k + 1) * P, :])
            n_rt = (N + P - 1) // P
            for rt in range(n_rt):
                rb = rt * P
                rsz = min(P, N - rb)
                xt_sb = sb.tile([P, KCH, P], BF16, tag="xt")
                for k in range(KCH):
                    nc.sync.dma_start(out=xt_sb[:, k, :rsz],
                                      in_=xT[k * P:(k + 1) * P, rb:rb + rsz])
                ph = ps.tile([P, fcols], F32, tag="ph")
                pa = ps.tile([P, 2 * nheads], F32, tag="pa")
                for k in range(KCH):
                    nc.tensor.matmul(out=ph[:rsz, :], lhsT=xt_sb[:, k, :rsz],
                                     rhs=wa_sb[:, k, :fcols],
                                     start=(k == 0), stop=(k == KCH - 1))
                    nc.tensor.matmul(out=pa[:rsz, :], lhsT=xt_sb[:, k, :rsz],
                                     rhs=wa_sb[:, k, fcols:],
                                     start=(k == 0), stop=(k == KCH - 1))
                hsb = sb.tile([P, ncolsP], BF16, tag="hsb")
                nc.scalar.activation(out=hsb[:rsz, :fcols], in_=ph[:rsz, :],
                                     func=mybir.ActivationFunctionType.Copy)
                nc.vector.tensor_copy(hsb[:rsz, fcols:ncols], pa[:rsz, :])
                if ncolsP > ncols:
                    nc.vector.memset(hsb[:rsz, ncols:], 0.0)
                nc.sync.dma_start(out=hpp[rb:rb + rsz, :], in_=hsb[:rsz, :])

            # ---- stage B: attention edge pass over this core's dst shard
            for t in range(NT):
                tb = t * TSZ
                si = sb.tile([P, C], I32, tag="si")
                nc.sync.dma_start(out=si[:], in_=srcidx[t])
                dc = sb.tile([P, C], F32, tag="dc")
                nc.sync.dma_start(out=dc[:], in_=dstcol[t])
                s0t = sb.tile([P, C, P], BF16, tag="s0t")
                nc.sync.dma_start(out=s0t[:], in_=s0tT[t])
                ad = sb.tile([P, nheads], BF16, tag="ad")
                nc.vector.memset(ad[:], 0.0)
                nc.sync.dma_start(out=ad[:TSZ, :],
                                  in_=hpp[tb:tb + TSZ, fcols + nheads:ncols])

                mst = stashp.tile([P, C, ncolsP], BF16, tag="mst")
                pden = psacc.tile([P, nheads], F32, tag="pden")
                padall = psacc.tile([P, C, nheads], F32, tag="padall")

                for c in range(C):
                    nc.gpsimd.indirect_dma_start(
                        out=mst[:, c, :], out_offset=None, in_=hpp[:, :],
                        in_offset=bass.IndirectOffsetOnAxis(ap=si[:, c:c + 1],
                                                            axis=0))
                # alpha_dst broadcast to edges: one small matmul per chunk
                for c in range(C):
                    nc.tensor.matmul(out=padall[:, c, :], lhsT=s0t[:, c, :],
                                     rhs=ad[:], start=True, stop=True)
                # tile-batched logits: pre -> leaky -> exp
                pre = stashp.tile([P, C, nheads], F32, tag="pre")
                nc.vector.tensor_tensor(out=pre[:],
                                        in0=mst[:, :, fcols:fcols + nheads],
                                        in1=padall[:], op=mybir.AluOpType.add)
                lg = stashp.tile([P, C, nheads], F32, tag="lg")
                nc.vector.tensor_scalar_mul(lg[:], pre[:], NEG)
                nc.vector.tensor_tensor(out=lg[:], in0=pre[:], in1=lg[:],
                                        op=mybir.AluOpType.max)
                expst = stashp.tile([P, C, nheads], F32, tag="expst")
                nc.scalar.activation(out=expst[:], in_=lg[:],
                                     func=mybir.ActivationFunctionType.Exp)
                expb = stashp.tile([P, C, nheads], BF16, tag="expb")
                nc.vector.tensor_copy(expb[:], expst[:])

                pout = psacc.tile([P, fcols], F32, tag="pout")
                for c in range(C):
                    s0 = sb.tile([P, P], BF16, tag="s0")
                    nc.vector.tensor_tensor(out=s0[:],
                                            in0=dc[:, c:c + 1].to_broadcast([P, P]),
                                            in1=iota_f[:],
                                            op=mybir.AluOpType.is_equal)
                    nc.tensor.matmul(out=pden[:], lhsT=s0[:],
                                     rhs=expb[:, c, :],
                                     start=(c == 0), stop=(c == C - 1))
                    mp = sb.tile([P, fcols], BF16, tag="mp")
                    for h in range(nheads):
                        eng = nc.vector if h % 2 == 0 else nc.scalar
                        if h % 2 == 0:
                            nc.vector.tensor_scalar_mul(
                                mp[:, h * HID:(h + 1) * HID],
                                mst[:, c, h * HID:(h + 1) * HID],
                                expst[:, c, h:h + 1])
                        else:
                            nc.scalar.activation(
                                out=mp[:, h * HID:(h + 1) * HID],
                                in_=mst[:, c, h * HID:(h + 1) * HID],
                                func=mybir.ActivationFunctionType.Copy,
                                scale=expst[:, c, h:h + 1])
                    nc.tensor.matmul(out=pout[:], lhsT=s0[:], rhs=mp[:],
                                     start=(c == 0), stop=(c == C - 1))

                dsum = sb.tile([P, nheads], F32, tag="dsum")
                nc.vector.tensor_scalar_add(dsum[:], pden[:], 1e-16)
                rden = sb.tile([P, nheads], F32, tag="rden")
                nc.vector.reciprocal(rden[:], dsum[:])
                osb = sb.tile([P, fcols], F32, tag="osb")
                for h in range(nheads):
                    nc.scalar.activation(out=osb[:, h * HID:(h + 1) * HID],
                                         in_=pout[:, h * HID:(h + 1) * HID],
                                         func=mybir.ActivationFunctionType.Relu,
                                         scale=rden[:, h:h + 1])
                nc.sync.dma_start(out=outsh[tb:tb + TSZ, :], in_=osb[:TSZ, :])

            # ---- pool + FC partial (launch 2 only)
            if do_pool:
                identf = constp.tile([G, G], F32)
                make_identity(nc, identf[:])
                iog = constp.tile([P, G], I32)
                nc.gpsimd.iota(iog[:], pattern=[[1, G]], base=0,
                               channel_multiplier=0)
                iogf = constp.tile([P, G], F32)
                nc.vector.tensor_copy(iogf[:], iog[:])
                fcw_sb = constp.tile([HID, OUT], F32)
                nc.sync.dma_start(out=fcw_sb[:], in_=fcw[:, :])
                rc_sb = constp.tile([G, 1], F32)
                nc.sync.dma_start(out=rc_sb[:], in_=rcounts[:, :])
                ppool = psacc.tile([G, HID], F32, tag="pden")
                for rt in range(NT):
                    rb = rt * P
                    rsz = min(P, NSH - rb)
                    h2t = sb.tile([P, HID], F32, tag="h2t")
                    if rsz < P:
                        nc.vector.memset(h2t[:], 0.0)
                    nc.sync.dma_start(out=h2t[:rsz, :], in_=outsh[rb:rb + rsz, :])
                    bc = sb.tile([P, 1], F32, tag="bc")
                    nc.sync.dma_start(out=bc[:], in_=batchcol[rb:rb + P, :])
                    b0 = sb.tile([P, G], F32, tag="b0")
                    nc.vector.tensor_tensor(out=b0[:],
                                            in0=bc[:].to_broadcast([P, G]),
                                            in1=iogf[:],
                                            op=mybir.AluOpType.is_equal)
                    nc.tensor.matmul(out=ppool[:], lhsT=b0[:], rhs=h2t[:],
                                     start=(rt == 0), stop=(rt == NT - 1))
                plsb = sb.tile([G, HID], F32, tag="plsb")
                nc.scalar.activation(out=plsb[:], in_=ppool[:],
                                     func=mybir.ActivationFunctionType.Copy,
                                     scale=rc_sb[:, :1])
                ptr2 = ps.tile([HID, G], F32, tag="ph")
                nc.tensor.transpose(out=ptr2[:], in_=plsb[:], identity=identf[:])
                plT = sb.tile([HID, G], F32, tag="plT")
                nc.vector.tensor_copy(plT[:], ptr2[:])
                py = ps.tile([G, OUT], F32, tag="pa")
                nc.tensor.matmul(out=py[:], lhsT=plT[:], rhs=fcw_sb[:],
                                 start=True, stop=True)
                ysb = sb.tile([G, OUT], F32, tag="ysb")
                nc.vector.tensor_copy(ysb[:], py[:])
                nc.sync.dma_start(out=yout[:, :], in_=ysb[:])

    nc.compile()
    return nc


def _edge_partition(src, dst):
    """Sort edges by dst, split per (core, tile), pad chunks. Returns
    (C, srcidx[NCORES,NT,P,C] int32 global-src, dstcol[NCORES,NT,P,C] f32)."""
    order = np.argsort(dst, kind="stable")
    ssrc, sdst = src[order], dst[order]
    counts = []
    bounds = []
    for cid in range(NCORES):
        for t in range(NT):
            lo = cid * NSH + t * TSZ
            hi = lo + TSZ
            a = np.searchsorted(sdst, lo, side="left")
            b = np.searchsorted(sdst, hi, side="left")
            bounds.append((a, b, lo))
            counts.append(b - a)
    C = int(max((cnt + P - 1) // P for cnt in counts))
    if C % 2:
        C += 1  # two equal half-tile gathers
    srcidx = np.zeros((NCORES, NT, P, C), np.int32)
    dstcol = np.full((NCORES, NT, P, C), -1.0, np.float32)
    k = 0
    for cid in range(NCORES):
        for t in range(NT):
            a, b, lo = bounds[k]
            k += 1
            ne = b - a
            s = np.zeros(C * P, np.int64)
            d = np.full(C * P, -1.0, np.float32)
            s[:ne] = ssrc[a:b]
            d[:ne] = (sdst[a:b] - lo).astype(np.float32)
            # roll src to core-local row order; edge j -> chunk j//P, part j%P
            s = (s - cid * NSH) % N
            srcidx[cid, t] = s.reshape(C, P).T
            dstcol[cid, t] = d.reshape(C, P).T
    d128 = np.arange(P, dtype=np.float32)
    s0tT = np.zeros((NCORES, NT, P, C * P), BF)
    for cid in range(NCORES):
        for t in range(NT):
            dcq = dstcol[cid, t]  # [P, C]
            oh = (dcq.T[:, None, :] == d128[None, :, None])  # [C, 128d, 128p]
            s0tT[cid, t] = oh.transpose(1, 0, 2).reshape(P, C * P).astype(BF)
    return C, srcidx, dstcol, s0tT


_cache = {}
_last = {}  # debug: last run's BassKernelResults per launch


def _run(nc, in_maps, label):
    import os
    kw = {}
    if os.environ.get("GAT_TRACE"):
        d = f"/tmp/gat_trace_{label}"
        os.makedirs(d, exist_ok=True)
        kw = dict(trace=True, tmpdir=d)
    res = run_bass_kernel_spmd(nc, in_maps, core_ids=list(range(NCORES)), **kw)
    _last[label] = res
    return res


def kernel(x, edge_index, batch, W1, a1s, a1d, b1, W2, a2s, a2d, b2, fc_w, fc_b):
    x = np.asarray(x, np.float32)
    edge_index = np.asarray(edge_index)
    batch = np.asarray(batch)
    W1 = np.asarray(W1, np.float32)
    W2 = np.asarray(W2, np.float32)
    fc_w = np.asarray(fc_w, np.float32)

    loops = np.arange(N, dtype=edge_index.dtype)
    src = np.concatenate([np.asarray(edge_index[0]), loops])
    dst = np.concatenate([np.asarray(edge_index[1]), loops])
    C, srcidx, dstcol, s0tT = _edge_partition(src, dst)

    # fold attention vectors into stage-A weights: A1 maps H1 -> [a_src|a_dst]
    A1 = np.zeros((HEADS * HID, 2 * HEADS), np.float32)
    for h in range(HEADS):
        A1[h * HID:(h + 1) * HID, h] = np.asarray(a1s, np.float32)[h]
        A1[h * HID:(h + 1) * HID, HEADS + h] = np.asarray(a1d, np.float32)[h]
    Wa1 = np.concatenate([W1, W1 @ A1], axis=1)  # [256, 520]
    A2 = np.zeros((HID, 2), np.float32)
    A2[:, 0] = np.asarray(a2s, np.float32)[0]
    A2[:, 1] = np.asarray(a2d, np.float32)[0]

    if ("l1", C) not in _cache:
        _cache[("l1", C)] = build_program(C, HEADS * HID, HEADS, IN, False)
    nc1 = _cache[("l1", C)]

    xT = np.ascontiguousarray(x.T).astype(BF)  # [256, 10000]
    Wa1 = Wa1.astype(BF)
    in_maps = []
    for cid in range(NCORES):
        roll = np.concatenate([xT[:, cid * NSH:], xT[:, :cid * NSH]], axis=1)
        in_maps.append(dict(xT=np.ascontiguousarray(roll), Wa=Wa1,
                            srcidx=srcidx[cid], dstcol=dstcol[cid],
                            s0tT=s0tT[cid]))
    res1 = _run(nc1, in_maps, "l1")
    l1 = np.concatenate([res1.results[cid]["outsh"] for cid in range(NCORES)],
                        axis=0)  # [10000, 512] relu'd

    Wa2 = np.concatenate([W2, W2 @ A2], axis=1)  # [512, 130]
    cnts = np.bincount(np.asarray(batch, np.int64), minlength=G).astype(np.float32)
    rcounts = (1.0 / np.maximum(cnts, 1.0)).reshape(G, 1)
    bc_full = np.asarray(batch, np.float32)

    if ("l2", C) not in _cache:
        _cache[("l2", C)] = build_program(C, HID, 1, HEADS * HID, True)
    nc2 = _cache[("l2", C)]

    l1T = np.ascontiguousarray(l1.T).astype(BF)  # [512, 10000]
    Wa2 = Wa2.astype(BF)
    in_maps2 = []
    for cid in range(NCORES):
        roll = np.concatenate([l1T[:, cid * NSH:], l1T[:, :cid * NSH]], axis=1)
        bcol = np.full((P * NT, 1), -1.0, np.float32)
        bcol[:NSH, 0] = bc_full[cid * NSH:(cid + 1) * NSH]
        in_maps2.append(dict(xT=np.ascontiguousarray(roll), Wa=Wa2,
                             srcidx=srcidx[cid], dstcol=dstcol[cid],
                             s0tT=s0tT[cid], batchcol=bcol, rcounts=rcounts,
                             fcw=fc_w))
    res2 = _run(nc2, in_maps2, "l2")
    y = sum(res2.results[cid]["yout"] for cid in range(NCORES))
    y = y + np.asarray(fc_b, np.float32)[None, :]
    return y.astype(np.float32)
